# revision 10
# baseline (speedup 1.0000x reference)
"""Trainium2 kernel for nn_LinearAutoDecoder (cluster-routed per-row 3x95 matvec).

out[i] = W[3*c_i : 3*c_i+3] @ x_i  with W = [W_pos | W_feat] in R^{384x95}.

Strategy: rows are grouped by cluster (each cluster's rows sharded round-robin
across the 8 cores so every core runs the identical static schedule), X is
streamed in a pre-transposed [96, R] layout, and the device does dense fp32r
matmuls (full PE rate at moving dim 512) with the per-cluster [96, 3]
stationary baked into the instruction stream as AP offsets. The host scatters
the [3, R] result back to original row order.
"""

import os
import sys

for _p in (
    "/root/.axon_site",
    "/root/.axon_site/_ro/trn_rl_repo",
    "/root/.axon_site/_ro/pypackages",
    "/opt/trn_rl_repo",
    "/opt/pypackages",
):
    if os.path.isdir(_p) and _p not in sys.path:
        sys.path.append(_p)

import numpy as np

N_CORES = 8
F = 95          # feature dim (63 pos + 32 latent)
FP = 96         # padded feature dim (matmul K)
NCL = 128       # clusters
ST = 512        # rows per supertile (matmul moving dim)
CH = 16         # supertiles per DMA chunk
USE_FP32R = True

_prog_cache = {}


def _build_program(schedule, R):
    from contextlib import ExitStack

    import concourse.bacc as bacc
    import concourse.tile as tile
    import concourse.tile_sem_assignment as tsa
    from concourse import mybir

    # Keep the end-of-kernel drain wait fan-in within walrus' per-instruction
    # sync-wait budget: two SWDGE completion lanes instead of eight.
    tsa.NUM_SWDGE_GLOBAL_SEMS = 2

    nc = bacc.Bacc(
        "TRN2", target_bir_lowering=False, debug=False, num_devices=N_CORES
    )
    xt = nc.dram_tensor("xt", [FP, R], mybir.dt.float32, kind="ExternalInput").ap()
    wt = nc.dram_tensor(
        "wt", [FP, 3 * NCL], mybir.dt.float32, kind="ExternalInput"
    ).ap()
    ot = nc.dram_tensor("ot", [3, R], mybir.dt.float32, kind="ExternalOutput").ap()

    T = len(schedule)
    assert T % CH == 0 and T * ST == R
    r_dt = mybir.dt.float32r if USE_FP32R else mybir.dt.float32

    with tile.TileContext(nc, trace_sim=False) as tc, ExitStack() as ctx:
        wpool = ctx.enter_context(tc.tile_pool(name="w", bufs=1))
        xpool = ctx.enter_context(tc.tile_pool(name="x", bufs=2))
        opool = ctx.enter_context(tc.tile_pool(name="o", bufs=2))
        ppool = ctx.enter_context(tc.tile_pool(name="p", bufs=4, space="PSUM"))

        w_sb = wpool.tile([FP, 3 * NCL], r_dt)
        nc.gpsimd.dma_start(w_sb[:], wt[:])

        for ch in range(T // CH):
            x_sb = xpool.tile([FP, CH * ST], r_dt)
            nc.gpsimd.dma_start(
                x_sb[:], xt[:, ch * CH * ST : (ch + 1) * CH * ST]
            )
            o_sb = opool.tile([3, CH * ST], mybir.dt.float32)
            for jp in range(CH // 2):
                ps = ppool.tile([3, 2 * ST], mybir.dt.float32)
                for h in range(2):
                    j = 2 * jp + h
                    c = schedule[ch * CH + j]
                    nc.tensor.matmul(
                        ps[:, h * ST : (h + 1) * ST],
                        lhsT=w_sb[:, 3 * c : 3 * c + 3],
                        rhs=x_sb[:, j * ST : (j + 1) * ST],
                        start=True,
                        stop=True,
                    )
                sl = slice(2 * jp * ST, (2 * jp + 2) * ST)
                if jp % 2 == 0:
                    nc.vector.tensor_copy(o_sb[:, sl], ps[:])
                else:
                    nc.scalar.copy(o_sb[:, sl], ps[:])
            nc.gpsimd.dma_start(
                ot[:, ch * CH * ST : (ch + 1) * CH * ST], o_sb[:]
            )
    nc.compile()
    return nc


def kernel(X, cluster_ids, W_pos, W_feat):
    X = np.asarray(X, dtype=np.float32)
    ids = np.asarray(cluster_ids, dtype=np.int32)
    W_pos = np.asarray(W_pos, dtype=np.float32)
    W_feat = np.asarray(W_feat, dtype=np.float32)
    N = X.shape[0]

    W = np.concatenate([W_pos, W_feat], axis=1)  # [384, 95]
    WT = np.zeros((FP, 3 * NCL), dtype=np.float32)
    WT[:F, :] = W.T  # column 3c+j = W[3c+j, :] (zero-padded K row 95)

    order = np.argsort(ids, kind="stable")
    counts = np.bincount(ids, minlength=NCL)
    offs = np.concatenate([[0], np.cumsum(counts)])
    Ks = [
        int(-(-(-(-int(counts[c]) // N_CORES)) // ST)) if counts[c] else 0
        for c in range(NCL)
    ]
    # Ks[c] = ceil(ceil(n_c / 8) / 512)
    Ks = [
        ((int(counts[c]) + N_CORES - 1) // N_CORES + ST - 1) // ST
        for c in range(NCL)
    ]
    schedule = [c for c in range(NCL) for _ in range(Ks[c])]
    while len(schedule) % CH:
        schedule.append(0)
    T = len(schedule)
    R = T * ST

    # Per-core row lists: cluster c's shard for core m is Ic[m::8], padded to
    # Ks[c]*512 with index N (an all-zero row appended to X).
    rows = np.full((N_CORES, R), N, dtype=np.int64)
    tile_base = 0
    for c in range(NCL):
        Ic = order[offs[c] : offs[c + 1]]
        for m in range(N_CORES):
            sh = Ic[m::N_CORES]
            rows[m, tile_base * ST : tile_base * ST + len(sh)] = sh
        tile_base += Ks[c]

    Xaug = np.zeros((N + 1, FP), dtype=np.float32)
    Xaug[:N, :F] = X

    in_maps = []
    for m in range(N_CORES):
        Xt = np.ascontiguousarray(Xaug[rows[m]].T)  # [96, R]
        in_maps.append({"xt": Xt, "wt": WT})

    key = (tuple(schedule), R)
    if key not in _prog_cache:
        _prog_cache.clear()
        _prog_cache[key] = _build_program(schedule, R)
    nc = _prog_cache[key]

    from concourse.bass_utils import run_bass_kernel_spmd

    res = run_bass_kernel_spmd(nc, in_maps, list(range(N_CORES)))

    out = np.zeros((N, 3), dtype=np.float32)
    for m in range(N_CORES):
        otm = res.results[m]["ot"]  # [3, R]
        valid = rows[m] != N
        out[rows[m][valid]] = otm.T[valid]
    return out
